# revision 47
# baseline (speedup 1.0000x reference)
"""Multi-head self-attention with RoPE on 8 Trainium2 NeuronCores.

Sharding: data-parallel over batch (2) x tensor-parallel over heads
(16 heads -> 4 groups of 4). Core c handles batch c//4, head group c%4.
Each core computes a partial output projection (d_in-sharded wo); the
4 partials per batch are summed on the host (the unshard step).

Design notes (~183us/core vs the 262-309us fp32r baseline):
  - Projection/score matmul operands are fp16 (PSUM accumulates fp32);
    et/Vs stay fp32r because fp16 activation *writes* cost ~+130ns/inst
    on the ACT engine. Halves weight loads and x/weight DMA bytes.
  - Emission order is only a scheduler priority; the tile list
    scheduler greedily fills PE idle slots with later-priority ready
    work. Projections of chunk b+1 and the late-emitted output
    projections act as PE filler inside the exp-latency windows of
    attention, keeping the PE DVFS p-state at full clock.
  - Both heads of an i-step share one 2-bank score tile -> one exp
    instruction per i-step (halves the ~310ns/inst ACT overhead).
  - PSUM budget (8 banks): pp x2 (proj/rope/outproj), sc x2x2-bank
    score pairs in flight, ops0/ops1 (per-head attention accumulators
    with the appended ones-row denominator).
  - Softmax normalization is split: the unnormalized accumulator and
    denominator row evacuate immediately at high priority (frees the
    ops banks in ~1.3us), while reciprocal -> broadcast -> multiply are
    deferred into the NEXT block's steady state. The broadcast runs as
    partition_broadcast on the otherwise-empty GpSimd engine; its
    microcode library is pre-warmed by a dummy broadcast in the DMA
    prologue shadow. (GpSimd reloads its library (~7us whole-device
    stall, invisible in traces) whenever it switches op classes -- with
    RoPE elementwise on DVE, broadcast is its only tensor op, so the
    library loads exactly once.)
  - qt/Vs evacuations run on the ACT engine (idle at block boundaries
    where DVE previously convoyed); RoPE elementwise runs on DVE in
    fp16 2x mode (~330ns vs ~1.15us per tile on GpSimd).
  - x chunks are double-buffered; chunk b+1's DMA is issued at the
    start of chunk b's emission; first-needed weights lead the DMA
    queues.
"""

import sys

for _p in ("/opt/trn_rl_repo", "/opt/pypackages"):
    if _p not in sys.path:
        sys.path.append(_p)

import numpy as np

import concourse.bass as bass
import concourse.mybir as mybir
import concourse.tile as tile
from concourse import bacc
from concourse.bass_utils import run_bass_kernel_spmd

# Problem constants (hardcoded per contract)
B = 2
S = 2048
DM = 1024
NH = 16
DK = 64
THETA = 10000.0
N_CORES = 8
HG = 4            # head groups (tensor-parallel)
HL = NH // HG     # heads per core = 4
DG = HL * DK      # group out dim = 256

P = 128
KO = DM // P      # 8 contraction subtiles for projections
MT = 2            # 128-row tiles of the 256-wide Q/K head-group dim
QB = 512          # q block width
NQB = S // QB     # 4
NKT = S // P      # 16 k tiles
F32 = mybir.dt.float32
F32R = mybir.dt.float32r
F16 = mybir.dt.float16


def _emit(ctx, tc, d):
    nc = tc.nc
    const = ctx.enter_context(tc.tile_pool(name="const", bufs=1))
    psum = ctx.enter_context(tc.tile_pool(name="psum", bufs=1, space="PSUM"))
    xpool = ctx.enter_context(tc.tile_pool(name="xpool", bufs=2))
    epool = ctx.enter_context(tc.tile_pool(name="epool", bufs=6))
    tpool = ctx.enter_context(tc.tile_pool(name="tpool", bufs=2))

    # ---- resident SBUF tensors ----
    wq_s = const.tile([P, KO, DG], F16)
    wk_s = const.tile([P, KO, DG], F16)
    wv_s = const.tile([P, KO, DG], F16)
    wo_s = const.tile([P, MT, DM], F16)
    cos_s = const.tile([P, S], F16)
    sin_s = const.tile([P, S], F16)
    rmat_s = const.tile([P, P], F16)
    ones_s = const.tile([1, DK], F16)
    ident_s = const.tile([P, P], F16)
    tri_s = const.tile([P, P], F16)
    Qp = const.tile([P, MT, S], F16)
    Kp = const.tile([P, MT, S], F16)
    Vs = const.tile([P, NKT, HL, DK + 2], F32R)
    As = const.tile([P, MT, S], F16)

    # first-needed tensors lead on the sync (HWDGE) queue; remaining
    # consts spread across the scalar/vector/gpsimd queues so no single
    # issue queue serializes the prologue
    nc.scalar.dma_start(wq_s[:], d["wqT"][:])
    nc.scalar.dma_start(rmat_s[:], d["rmat"][:])
    nc.scalar.dma_start(cos_s[:], d["cosd"][:])
    nc.scalar.dma_start(wk_s[:], d["wkT"][:])
    nc.gpsimd.dma_start(sin_s[:], d["sind"][:])
    nc.gpsimd.dma_start(wv_s[:], d["wvT"][:])
    nc.gpsimd.dma_start(tri_s[:], d["trimask"][:])
    nc.gpsimd.dma_start(ident_s[:], d["ident"][:])
    nc.gpsimd.dma_start(wo_s[:], d["woT"][:])
    # ones column for the denominator rows
    nc.gpsimd.memset(Vs[:, :, :, DK : DK + 1].bitcast(F32), 1.0)
    nc.gpsimd.memset(ones_s[:], 1.0)
    # Pool runs ONLY partition_broadcast as a tensor op now (RoPE moved
    # to DVE) -- warm its microcode library during the DMA prologue so
    # the first real broadcast doesn't pay the ~7us reload mid-kernel
    wrm = const.tile([1, DK], F32, name="wrm")
    nc.gpsimd.memset(wrm[:], 1.0)
    wrmo = const.tile([4, DK], F32, name="wrmo")
    nc.gpsimd.partition_broadcast(wrmo[:], wrm[:], channels=4)

    xc_tiles = {}

    def fetch_x(b, parts=2):
        xc = xpool.tile([P, KO, QB], F16, tag="xc", name="xc")
        kq = KO // parts
        for q in range(parts):
            nc.sync.dma_start(
                xc[:, q * kq : (q + 1) * kq, :],
                d["xT"][b, :, q * kq : (q + 1) * kq, :],
            )
        xc_tiles[b] = xc

    fetch_x(0)

    def emit_proj(b):
        """Q/K projections + RoPE and V projection for 512-col chunk b."""
        if b + 1 < NQB:
            fetch_x(b + 1)
        xc = xc_tiles[b]
        cols = slice(b * QB, (b + 1) * QB)
        for w_s, dst in ((wq_s, Qp), (wk_s, Kp)):
            for mt in range(MT):
                ps = psum.tile([P, QB], F32, tag="pp", bufs=2, name="pp")
                for ko in range(KO):
                    nc.tensor.matmul(
                        ps[:],
                        lhsT=(w_s[:, ko, mt * P : (mt + 1) * P]),
                        rhs=(xc[:, ko, :]),
                        start=(ko == 0),
                        stop=(ko == KO - 1),
                    )
                qt = tpool.tile([P, QB], F16, tag="qt", bufs=6, name="qt")
                nc.scalar.activation(qt[:], ps[:],
                                     mybir.ActivationFunctionType.Copy)
                ps2 = psum.tile([P, QB], F32, tag="pp", bufs=2, name="pp")
                nc.tensor.matmul(
                    ps2[:], lhsT=(rmat_s[:]), rhs=(qt[:]), start=True, stop=True
                )
                tsin = tpool.tile([P, QB], F16, tag="tsin", bufs=6, name="tsin")
                nc.vector.tensor_mul(tsin[:], ps2[:], sin_s[:, cols])
                nc.vector.tensor_mul(dst[:, mt, cols], qt[:], cos_s[:, cols])
                nc.vector.tensor_add(dst[:, mt, cols], dst[:, mt, cols], tsin[:])
        for st in range(4 * b, 4 * b + 4):
            psv = psum.tile([P, HL, DK], F32, tag="pp", bufs=2, name="ppv")
            for ko in range(KO):
                nc.tensor.matmul(
                    psv[:],
                    lhsT=(xc[:, ko, (st % 4) * P : (st % 4 + 1) * P]),
                    rhs=(wv_s[:, ko, :]),
                    start=(ko == 0),
                    stop=(ko == KO - 1),
                )
            nc.scalar.activation(Vs[:, st, :, 0:DK], psv[:],
                                 mybir.ActivationFunctionType.Copy)

    pending_norm = []

    def flush_norm(tag="sc", hp=False):
        for fin in pending_norm:
            fin(tag, hp)
        pending_norm.clear()

    def emit_attn(j):
        """Causal attention for q block j (needs Qp/Kp/Vs chunks <= j)."""
        jcols = slice(j * QB, (j + 1) * QB)
        for mt in range(MT):
            hpair = (2 * mt, 2 * mt + 1)
            with nc.named_scope(f"attn{j}_{mt}"):
                ops = {
                    h: psum.tile([P, QB], F32, tag=f"ops{h % 2}", bufs=1,
                                 name=f"ops{h % 2}")
                    for h in hpair
                }
                if mt == 1:
                    flush_norm()
                for i in range(4 * j + 4):
                    c0 = P * (i - 4 * j) if i >= 4 * j else 0
                    # both heads' scores in one 2-bank tile -> one exp
                    sp = psum.tile([P, 2, QB], F32, tag="sc", bufs=2, name="sp")
                    for hh, h in enumerate(hpair):
                        pb = DK * (h % 2)
                        nc.tensor.matmul(
                            sp[:, hh, c0:QB],
                            lhsT=(Kp[pb : pb + DK, mt, i * P : (i + 1) * P]),
                            rhs=(Qp[pb : pb + DK, mt,
                                    j * QB + c0 : (j + 1) * QB]),
                            start=True,
                            stop=True,
                        )
                    if i >= 4 * j:
                        # causal mask accumulated by the PE itself:
                        # sp[:, hh, diag] += I^T @ (-60000 upper triangle);
                        # keeps the score->exp chain on one engine
                        for hh in range(2):
                            nc.tensor.matmul(
                                sp[:, hh, c0 : c0 + P],
                                lhsT=ident_s[:],
                                rhs=tri_s[:],
                                start=False,
                                stop=True,
                                skip_group_check=True,
                            )
                    et = epool.tile([P, 2, QB], F32R, tag="et", bufs=8, name="et")
                    nc.scalar.activation(
                        et[:, :, c0:QB], sp[:, :, c0:QB],
                        mybir.ActivationFunctionType.Exp,
                    )
                    for hh, h in enumerate(hpair):
                        nc.tensor.matmul(
                            ops[h][0 : DK + 1, c0:QB],
                            lhsT=(Vs[:, i, h, 0 : DK + 1]),
                            rhs=(et[:, hh, c0:QB]),
                            start=(i == 0),
                            stop=(i == 4 * j + 3),
                        )
                for h in hpair:
                    pb = DK * (h % 2)
                    with tc.high_priority():
                        oc = tpool.tile([DK, QB], F32, tag="oc", bufs=6,
                                        name="oc")
                        nc.vector.tensor_copy(oc[:], ops[h][0:DK, :])
                        rrow = tpool.tile([1, QB], F32, tag="rrow", bufs=6,
                                          name="rrow")
                        nc.vector.tensor_copy(rrow[:], ops[h][DK : DK + 1, :])

                    def finalize(tag, hp=False, pb=pb, mt=mt, jcols=jcols,
                                 oc=oc, rrow=rrow):
                        from contextlib import nullcontext
                        with tc.high_priority() if hp else nullcontext():
                            nc.vector.reciprocal_approx_fast(rrow[:], rrow[:])
                            rb = tpool.tile([DK, QB], F32, tag="rb", bufs=4,
                                            name="rb")
                            nc.gpsimd.partition_broadcast(rb[:], rrow[:],
                                                          channels=DK)
                            nc.vector.tensor_mul(
                                As[pb : pb + DK, mt, jcols], oc[:], rb[:]
                            )

                    pending_norm.append(finalize)

    def emit_outproj(j):
        for st in range(4 * j, 4 * j + 4):
            for nh2 in range(2):
                ncols = slice(nh2 * QB, (nh2 + 1) * QB)
                yps = psum.tile([P, QB], F32, tag="pp", bufs=2, name="ypp")
                for p_ in range(MT):
                    nc.tensor.matmul(
                        yps[:],
                        lhsT=(As[:, p_, st * P : (st + 1) * P]),
                        rhs=(wo_s[:, p_, ncols]),
                        start=(p_ == 0),
                        stop=(p_ == MT - 1),
                    )
                ysb = tpool.tile([P, QB], F32, tag="ysb", bufs=4, name="ysb")
                nc.vector.tensor_copy(ysb[:], yps[:])
                nc.sync.dma_start(d["y"][st, nh2], ysb[:])

    for b in range(NQB):
        with nc.named_scope(f"proj{b}"):
            emit_proj(b)
        emit_attn(b)
        if b >= 2:
            with nc.named_scope(f"oproj{b - 2}"):
                emit_outproj(b - 2)
    flush_norm(hp=True)
    for j in (2, 3):
        with nc.named_scope(f"oproj{j}"):
            emit_outproj(j)


def _build():
    nc = bacc.Bacc("TRN2", target_bir_lowering=False, debug=False,
                   num_devices=N_CORES)
    d = {}
    d["xT"] = nc.dram_tensor("xT", [NQB, P, KO, QB], F16, kind="ExternalInput").ap()
    d["wqT"] = nc.dram_tensor("wqT", [P, KO, DG], F16, kind="ExternalInput").ap()
    d["wkT"] = nc.dram_tensor("wkT", [P, KO, DG], F16, kind="ExternalInput").ap()
    d["wvT"] = nc.dram_tensor("wvT", [P, KO, DG], F16, kind="ExternalInput").ap()
    d["woT"] = nc.dram_tensor("woT", [P, MT, DM], F16, kind="ExternalInput").ap()
    d["cosd"] = nc.dram_tensor("cosd", [P, S], F16, kind="ExternalInput").ap()
    d["sind"] = nc.dram_tensor("sind", [P, S], F16, kind="ExternalInput").ap()
    d["rmat"] = nc.dram_tensor("rmat", [P, P], F16, kind="ExternalInput").ap()
    d["trimask"] = nc.dram_tensor("trimask", [P, P], F16, kind="ExternalInput").ap()
    d["ident"] = nc.dram_tensor("ident", [P, P], F16, kind="ExternalInput").ap()
    d["y"] = nc.dram_tensor("y", [NKT, 2, P, QB], F32, kind="ExternalOutput").ap()
    from contextlib import ExitStack
    with tile.TileContext(nc) as tc, ExitStack() as ctx:
        _emit(ctx, tc, d)
    nc.compile()
    return nc


_cache = {}


def _get_nc():
    if "nc" not in _cache:
        _cache["nc"] = _build()
    return _cache["nc"]


def _host_prep(x, token_positions, wq, wk, wv, wo):
    x = np.asarray(x, dtype=np.float32)
    pos = np.asarray(token_positions, dtype=np.float32)
    wq = np.asarray(wq, dtype=np.float32)
    wk = np.asarray(wk, dtype=np.float32)
    wv = np.asarray(wv, dtype=np.float32)
    wo = np.asarray(wo, dtype=np.float32)

    freqs = 1.0 / THETA ** (np.arange(0, DK, 2, dtype=np.float32) / DK)  # (32,)
    ang = pos[:, None] * freqs[None, :]          # (S, 32)
    cos_t, sin_t = np.cos(ang), np.sin(ang)       # (S, 32)
    jmap = (np.arange(P) % DK) // 2               # row -> freq index
    cosd = np.ascontiguousarray(cos_t.T[jmap, :]).astype(np.float16)  # (128, S)
    sind = np.ascontiguousarray(sin_t.T[jmap, :]).astype(np.float16)

    rmat = np.zeros((P, P), dtype=np.float32)
    m = np.arange(0, P, 2)
    rmat[m + 1, m] = -1.0   # out[2m]   = -in[2m+1]
    rmat[m, m + 1] = 1.0    # out[2m+1] =  in[2m]
    rmat = rmat.astype(np.float16)

    tri = np.where(
        np.arange(P)[:, None] <= np.arange(P)[None, :], 0.0, -60000.0
    ).astype(np.float16)
    ident = np.eye(P, dtype=np.float16)

    def tile3(a2d, inner=P):
        # [K, M] -> [inner, K//inner, M] with K = ko*inner + ki
        K, M = a2d.shape
        return np.ascontiguousarray(
            a2d.reshape(K // inner, inner, M).transpose(1, 0, 2)
        )

    in_maps = []
    scale = 1.0 / np.sqrt(np.float32(DK))
    for c in range(N_CORES):
        b, g = divmod(c, HG)
        gs = slice(g * DG, (g + 1) * DG)
        xT = np.ascontiguousarray(
            tile3(x[b].T).reshape(P, KO, NQB, QB).transpose(2, 0, 1, 3)
        ).astype(np.float16)                               # [4, 128, 8, 512]
        wqT = tile3((wq[gs] * scale).T.copy()).astype(np.float16)  # [128, 8, 256]
        wkT = tile3(wk[gs].T.copy()).astype(np.float16)
        wvT = tile3(wv[gs].T.copy()).astype(np.float16)
        woT = tile3(wo[:, gs].T.copy()).astype(np.float16)  # [128, 2, 1024]
        in_maps.append({
            "xT": xT, "wqT": wqT, "wkT": wkT, "wvT": wvT, "woT": woT,
            "cosd": cosd, "sind": sind, "rmat": rmat, "trimask": tri,
            "ident": ident,
        })
    return in_maps


def run(x, token_positions, wq, wk, wv, wo, trace=False):
    nc = _get_nc()
    in_maps = _host_prep(x, token_positions, wq, wk, wv, wo)
    res = run_bass_kernel_spmd(nc, in_maps, list(range(N_CORES)), trace=trace)
    y = np.zeros((B, S, DM), dtype=np.float32)
    for c in range(N_CORES):
        blk = res.results[c]["y"]  # [NKT, 2, 128, 512]
        y[c // HG] += blk.transpose(0, 2, 1, 3).reshape(S, DM)
    return y, res


def kernel(x, token_positions, wq, wk, wv, wo):
    y, _ = run(x, token_positions, wq, wk, wv, wo)
    return y
